# revision 15
# baseline (speedup 1.0000x reference)
"""Trainium2 Bass kernel for nn_CompactLoss_13864154431845.

Loss (from the reference, with the clip being a no-op for randn data):
    loss = mean_b [ (1/G) * sum_g ||x_{b,g} - c_g||^2 ]
         = (SSQ - 2*CROSS + B * CSQ) / (B*G)
where
    SSQ   = sum_{g,b,d} x^2                    (global sum of squares)
    CROSS = sum_g s_g . c_g,  s_g = sum_b x[g,b,:]   (per-group column sums)
    CSQ   = sum_g ||c_g||^2,  c_g = L2-normalized centers rows

Device work (memory-bound, one pass over the 1 GiB input):
  - shard batch across 8 cores (4096 rows each)
  - HWDGE (sync) streams 2 MiB f32 supertiles at HBM line rate; the last
    supertile is split into 4 x 512 KiB chunks so the drain chain after the
    final byte is short
  - per tile (128 rows x 512 cols):
      ACT: f32r -> bf16 copy (halves PE matmul passes)
      PE:  indicator-matmul accumulates column sums of group g into row g of
           a single (16,512) PSUM tile (one accumulation group for the whole
           kernel -- this HW path only honors the first start_tensor_calc)
      DVE: bn_stats -> (count, mean, M2) per partition on the exact f32 data
  - per group: one small bn_aggr over that group's 32 tile-stats, scheduled
    inside the stream so no large aggregate sits on the drain critical path
  - outputs per core: s (16,512) column sums on the ACT ring, mv (128,16,2)
    per-group mean/var on the sync ring (issue slots overlap)
Host: combine in float64, fold in centers, return float32 scalar.
"""

import sys

sys.path.insert(0, "/opt/trn_rl_repo")

from contextlib import ExitStack

import numpy as np

import concourse.bacc as bacc
import concourse.tile as tile
from concourse import mybir
from concourse.bass_utils import run_bass_kernel_spmd

G = 16
B = 32768
D = 512
P = 128
N_CORES = 8
BS = B // N_CORES          # 4096 rows per core
NT = BS // P               # 32 row-tiles per (core, group)
ST = 8                     # 512-col chunks per supertile; partition p holds rows 8p..8p+7
NST = NT // ST             # supertiles per group (2 MiB DMAs, 16 KiB/partition contiguous)
TILES_PER_CORE = G * NT    # 512
# mean/var chunks: one per group, except the last group is split 24+8 so the
# final aggregate (and the mv_out DMA behind it) sits on a short drain chain
CHUNK_TILES = [NT] * (G - 1) + [NT - 8, 8]
N_SLOTS = len(CHUNK_TILES)  # 17

_CACHE = {}


def _build(trace=False):
    key = "nc"
    if key in _CACHE:
        return _CACHE[key]

    F32R = mybir.dt.float32r
    nc = bacc.Bacc("TRN2", target_bir_lowering=False, debug=False)
    x = nc.dram_tensor("x", [G, BS, D], F32R, kind="ExternalInput").ap()
    ind_d = nc.dram_tensor("ind", [P, G, G], mybir.dt.bfloat16, kind="ExternalInput").ap()
    s_out = nc.dram_tensor("s_out", [G, D], mybir.dt.float32, kind="ExternalOutput").ap()
    mv_out = nc.dram_tensor("mv_out", [P, N_SLOTS, 2], mybir.dt.float32, kind="ExternalOutput").ap()

    with tile.TileContext(nc) as tc:
        with ExitStack() as ctx:
            singles = ctx.enter_context(tc.tile_pool(name="singles", bufs=1))
            xpool = ctx.enter_context(tc.tile_pool(name="xp", bufs=6))
            xbpool = ctx.enter_context(tc.tile_pool(name="xb", bufs=3))
            psum = ctx.enter_context(tc.tile_pool(name="psum", bufs=1, space="PSUM"))

            # indicator stationaries: ind[:, g, :] is (128, G) with column g = 1
            # (host-provided: DVE memset rejects f32r, and f32r matmuls need
            # both operands f32r)
            ind = singles.tile([P, G, G], mybir.dt.bfloat16)
            nc.scalar.dma_start(out=ind, in_=ind_d)  # ACT ring: keep SP free for x

            stats = singles.tile([P, TILES_PER_CORE, 6], mybir.dt.float32)
            mv = singles.tile([P, N_SLOTS, 2], mybir.dt.float32)
            ps = psum.tile([G, D], mybir.dt.float32)  # one bank, partitions 0..15
            s_sb = singles.tile([G, D], mybir.dt.float32)

            n_mm = 0
            total_mm = TILES_PER_CORE

            def do_chunk(g, dram_slice, t0, n_tiles):
                """One DMA of n_tiles (128,512) tiles for group g starting at
                per-group tile index t0, then copy/matmul/stats per tile."""
                nonlocal n_mm
                xt = xpool.tile([P, n_tiles, D], F32R)
                nc.sync.dma_start(out=xt, in_=dram_slice)
                # bf16 copy for the PE (halves matmul passes); exact-path
                # stats stay on the f32r data
                xb = xbpool.tile([P, n_tiles, D], mybir.dt.bfloat16)
                nc.scalar.copy(xb, xt)
                for j in range(n_tiles):
                    nc.tensor.matmul(
                        ps[0:G, :],
                        ind[:, g, :],
                        xb[:, j, :],
                        start=(n_mm == 0),
                        stop=(n_mm == total_mm - 1),
                        skip_group_check=True,
                    )
                    n_mm += 1
                    nc.vector.bn_stats(out=stats[:, g * NT + t0 + j, :], in_=xt[:, j, :])

            for g in range(G):
                # supertile s = 1024 consecutive rows; partition p takes rows
                # s*1024 + 8p .. +7 -> one contiguous 16 KiB descriptor per
                # partition (DMA efficiency), harmless row permutation for
                # column sums and global stats
                xg = x[g].rearrange("(s p j) d -> s p j d", p=P, j=ST)  # (NST,128,8,512)
                last_group = g == G - 1
                for st in range(NST - 1 if last_group else NST):
                    do_chunk(g, xg[st], st * ST, ST)
                if not last_group:
                    # per-group aggregate (192 elem/partition, ~1us) runs
                    # inside the stream, off the drain critical path
                    nc.vector.bn_aggr(
                        out=mv[:, g, :], in_=stats[:, g * NT : (g + 1) * NT, :]
                    )
                else:
                    # aggregate the first 24 tiles as soon as they're in
                    nc.vector.bn_aggr(
                        out=mv[:, g, :], in_=stats[:, g * NT : g * NT + NT - 8, :]
                    )
                    # stream the final supertile as 4 x 512 KiB chunks so the
                    # post-stream chain (ACT copy -> matmuls -> psum drain,
                    # and bn_stats -> mini-aggr -> mv_out) is shallow
                    xq = x[g].rearrange("(s p j) d -> s p j d", p=P, j=2)
                    for q in range(4):
                        do_chunk(g, xq[4 * (NST - 1) + q], (NST - 1) * ST + 2 * q, 2)
                    nc.vector.bn_aggr(
                        out=mv[:, G, :], in_=stats[:, g * NT + NT - 8 :, :]
                    )

            # drain: psum -> sbuf on ACT (queued behind the last bf16 copy),
            # outputs on separate rings so the two issue slots overlap
            nc.scalar.copy(s_sb, ps)
            nc.scalar.dma_start(out=s_out, in_=s_sb)
            nc.sync.dma_start(out=mv_out, in_=mv)

    nc.compile()
    _CACHE[key] = nc
    return nc


def _make_ind():
    import ml_dtypes
    ind = np.zeros((P, G, G), dtype=ml_dtypes.bfloat16)
    for g in range(G):
        ind[:, g, g] = 1.0
    return ind


def _run_device(group_feats, trace=False):
    nc = _build()
    ind = _make_ind()
    in_maps = []
    for c in range(N_CORES):
        shard = np.ascontiguousarray(group_feats[:, c * BS : (c + 1) * BS, :])
        in_maps.append({"x": shard, "ind": ind})
    res = run_bass_kernel_spmd(nc, in_maps, list(range(N_CORES)), trace=trace)
    return res


def kernel(group_feats, centers, _trace=False, _return_res=False):
    group_feats = np.asarray(group_feats, dtype=np.float32)
    centers = np.asarray(centers, dtype=np.float32)

    res = _run_device(group_feats, trace=_trace)

    s_total = np.zeros((G, D), dtype=np.float64)
    ssq_total = 0.0
    n_slot = np.array(CHUNK_TILES, dtype=np.float64) * D  # elems/partition/slot
    for c in range(N_CORES):
        s_total += res.results[c]["s_out"].astype(np.float64)
        mv = res.results[c]["mv_out"].astype(np.float64)  # (P, N_SLOTS, 2)
        ssq_total += (n_slot[None, :] * (mv[:, :, 1] + mv[:, :, 0] ** 2)).sum()

    c64 = centers.astype(np.float64)
    norm = np.sqrt((c64 * c64).sum(axis=1, keepdims=True))
    c_hat = c64 / np.maximum(norm, 1e-12)
    cross = float((s_total * c_hat).sum())
    csq = float((c_hat * c_hat).sum())

    loss = (ssq_total - 2.0 * cross + B * csq) / (B * G)
    out = np.float32(loss)
    if _return_res:
        return out, res
    return out
